# revision 32
# baseline (speedup 1.0000x reference)
"""Distributed causal self-attention kernel for 8 TRN2 NeuronCores (Bass/Tile).

Self-contained: kernel(**inputs) takes the FULL unsharded inputs
(x [2,4096,768], W_kqv [12,768,192], W_proj [768,768], b_proj [768]),
shards them across 8 cores (batch x head-group), runs one SPMD NEFF via
bass_utils.run_bass_kernel_spmd, and reassembles the full [2,4096,768] output.

v2: software-pipelined phases — QKV projection interleaved with A/B-head
attention per token-half so scalar-engine exp overlaps projection matmuls;
V projected directly into [keys, vdim] layout (no PE transposes); PV
stationaries are 128-col (FWL); fast approximate reciprocal; head-C pass
runs under the A/B all-to-alls so only the small head-C collective is
tail-exposed.
"""

import sys

for p in ("/opt/trn_rl_repo", "/root/.axon_site/_ro/trn_rl_repo"):
    if p not in sys.path:
        sys.path.insert(0, p)


import ml_dtypes
import numpy as np

import concourse.bass as bass
import concourse.mybir as mybir
import concourse.tile as tile

F32 = mybir.dt.float32
F32R = mybir.dt.float32r
BF16 = mybir.dt.bfloat16
EXPF = mybir.ActivationFunctionType.Exp


class Cfg:
    def __init__(self, N=4096, D=768, H=12, B=2, NCORES=8):
        self.N, self.D, self.H, self.B, self.NCORES = N, D, H, B, NCORES
        self.HD = D // H          # 64
        self.HPC = H // (NCORES // B)   # heads per core = 3
        self.KB = 128             # k block
        self.NKB = N // self.KB   # k blocks (32)
        self.QT = N // NCORES     # q tile == per-rank token chunk (512)
        self.R = self.QT // self.KB  # diag masks per q tile (4)
        self.KC = D // 128        # contraction chunks (6)
        self.VW = 65              # v block cols: 64 v dims + ones col
        self.VS = 3 * self.VW     # v_sb stride per key block (195)
        assert self.HD == 64 and self.HPC == 3 and self.QT % self.KB == 0


def r32(ap):
    return ap.bitcast(F32R) if ap.dtype == F32 else ap


def build(tc: tile.TileContext, out_y: bass.AP, ins: dict, cfg: Cfg):
    nc = tc.nc
    ctx_lp = nc.allow_low_precision(reason="fp32r matmul pipeline")
    ctx_lp.__enter__()
    N, D, QT, KB, R, KC, NKB = cfg.N, cfg.D, cfg.QT, cfg.KB, cfg.R, cfg.KC, cfg.NKB
    HD, VW, VS = cfg.HD, cfg.VW, cfg.VS
    scale = 1.0 / np.sqrt(HD)
    xT, wkv, wpt, bias = ins["xT"], ins["wkv"], ins["wpt"], ins["bias"]

    persist = tc.alloc_tile_pool(name="persist", bufs=1)
    const_p = persist

    # ---- weights + x. Few, large DMAs: the sync engine issues descriptors
    # serially (~0.6us each), so batching directly shortens startup.
    wkv_sb = []   # [weff(512) | wv(192)] per contraction chunk
    for kc in range(KC):
        w = const_p.tile([128, 704], BF16, name=f"wkv{kc}")
        nc.sync.dma_start(w[:], wkv[128 * kc:128 * (kc + 1), :])
        wkv_sb.append(w)
    xt_sb = []
    for kc in range(KC):
        x = const_p.tile([128, N], BF16, name=f"xt{kc}")
        nc.sync.dma_start(x[:, 0:1024], xT[128 * kc:128 * (kc + 1), 0:1024])
        xt_sb.append(x)
    bias_sb = const_p.tile([1, D], F32R)
    nc.sync.dma_start(bias_sb[:], bias[:].bitcast(F32R))
    # rest of x + the phase-D weights in the background
    for kc in range(KC):
        nc.sync.dma_start(
            xt_sb[kc][:, 1024:N], xT[128 * kc:128 * (kc + 1), 1024:N])
    wp_sb = []
    for kc in range(KC):
        w = const_p.tile([128, D], BF16, name=f"wp{kc}")
        nc.sync.dma_start(w[:], wpt[128 * kc:128 * (kc + 1), :])
        wp_sb.append(w)

    # diag masks: mask_d[p, c] = 1.0 if c >= KB*d + p else 0
    masks = []
    for d in range(R):
        mk = const_p.tile([128, QT], BF16, name=f"mask{d}")
        nc.vector.memset(mk[:], 1.0)
        nc.gpsimd.affine_select(
            out=mk[:], in_=mk[:], compare_op=mybir.AluOpType.is_ge,
            fill=0.0, base=-KB * d, pattern=[[1, QT]], channel_multiplier=-1)
        masks.append(mk)

    ones128 = const_p.tile([1, 128], F32R)
    onestage = const_p.tile([1, 128], F32)
    nc.vector.memset(onestage[:], 1.0)
    nc.vector.tensor_copy(ones128[:], onestage[:])
    # 64-partition ones / zero-padded bias pair: lets the phase-D bias add run
    # as a (64,128)-tile matmul matching the lo-half group's tile config
    ones64 = const_p.tile([64, 128], BF16)
    nc.vector.memset(ones64[:], 1.0)
    bias64 = const_p.tile([64, D], BF16)
    nc.vector.memset(bias64[:], 0.0)
    nc.vector.tensor_copy(bias64[0:1, :], bias_sb[:].bitcast(F32))

    # v in [keys, vdim] layout: per key block j cols VS*j + VW*hi + [0:64]
    # are head hi's v, col VS*j + VW*hi + 64 is the ones column. Tail pad so
    # 128-col stationary reads (FWL) stay in bounds. memset 1.0 fills the
    # ones columns; v copies overwrite the data columns.
    v_sb = const_p.tile([128, VS * NKB + 128], BF16)
    # quarter memsets so early v copies only wait on their own region
    VQ = (VS * NKB + 128) // 4
    for q in range(4):
        hi_col = VS * NKB + 128 if q == 3 else VQ * (q + 1)
        nc.vector.memset(v_sb[:, VQ * q:hi_col], 1.0)

    # persistent kq activations
    kq_ab = const_p.tile([128, 2 * N], BF16)   # p0:64 kA|qA, p64:128 kB|qB
    kq_ck = const_p.tile([128, N], BF16)       # kC duplicated in both halves
    kq_cq = const_p.tile([128, N], BF16)       # qC duplicated in both halves

    dram = tc.alloc_tile_pool(name="dram", bufs=1, space="DRAM")
    a2a_in = []
    a2a_out = []
    for hi in range(3):
        ai = dram.tile([cfg.NCORES, 64, QT], BF16, name=f"a2ain{hi}")
        ao = dram.tile([cfg.NCORES, 64, QT], BF16, name=f"a2aout{hi}")
        a2a_in.append(ai)
        a2a_out.append(ao)

    def k_slice(hi, j):
        if hi == 0:
            return kq_ab[0:64, KB * j:KB * (j + 1)]
        if hi == 1:
            return kq_ab[64:128, KB * j:KB * (j + 1)]
        return kq_ck[0:64, KB * j:KB * (j + 1)]

    def q_slice(hi, qt):
        if hi == 0:
            return kq_ab[0:64, N + QT * qt:N + QT * (qt + 1)]
        if hi == 1:
            return kq_ab[64:128, N + QT * qt:N + QT * (qt + 1)]
        return kq_cq[0:64, QT * qt:QT * (qt + 1)]

    def v_stat(hi, j):
        # 128-col stationary: cols 0:64 = v, col 64 = ones, rest junk
        return v_sb[:, VS * j + VW * hi:VS * j + VW * hi + 128]

    with (
        tc.tile_pool(name="scr_ps", bufs=2, space="PSUM") as scr_psp,
        tc.tile_pool(name="s_ps", bufs=2, space="PSUM") as s_psp,
        tc.tile_pool(name="ctx_ps", bufs=1, space="PSUM") as ctx_psp,
        tc.tile_pool(name="exp_sb", bufs=3) as exp_sbp,
        tc.tile_pool(name="small_sb", bufs=3) as small_p,
        tc.tile_pool(name="cn_sb", bufs=3) as cn_p,
    ):
        def kq_dst(mt, gch):
            fr = slice(512 * gch, 512 * (gch + 1))
            if mt == 0:
                return kq_ab[:, fr]
            if mt == 1:
                return kq_ab[:, N + 512 * gch:N + 512 * (gch + 1)]
            if mt == 2:
                return kq_ck[:, fr]
            return kq_cq[:, fr]

        def kq_tile(mt, gch):
            fr = slice(512 * gch, 512 * (gch + 1))
            ps = scr_psp.tile([128, 512], F32, name="scr")
            for kc in range(KC):
                nc.tensor.matmul(
                    ps[:], wkv_sb[kc][:, 128 * mt:128 * (mt + 1)],
                    xt_sb[kc][:, fr],
                    start=(kc == 0), stop=(kc == KC - 1))
            nc.vector.tensor_copy(kq_dst(mt, gch), ps[:])

        def v_tile(j):
            tfr = slice(128 * j, 128 * (j + 1))
            ps = scr_psp.tile([128, 512], F32, name="scr")
            for kc in range(KC):
                nc.tensor.matmul(
                    ps[:, 0:192], xt_sb[kc][:, tfr], wkv_sb[kc][:, 512:704],
                    start=(kc == 0), stop=(kc == KC - 1))
            for hi in range(3):
                nc.vector.tensor_copy(
                    v_sb[:, VS * j + VW * hi:VS * j + VW * hi + 64],
                    ps[:, 64 * hi:64 * (hi + 1)])

        def a_chunk_tiles(gch):
            """Projection work for one 512-token chunk as 8 filler closures."""
            fillers = [
                (lambda mt=mt: kq_tile(mt, gch)) for mt in range(4)]
            fillers += [
                (lambda j=j: v_tile(j)) for j in range(4 * gch, 4 * gch + 4)]
            return fillers

        def norm_and_ship(hi, qt, ctx_ps):
            # custom DVE ops misread partition-offset inputs: stage the sums
            # row at partition 0 before the approx reciprocal
            s_f32 = small_p.tile([1, QT], F32, name="s_f32")
            nc.vector.tensor_copy(s_f32[:], ctx_ps[64:65, :])
            r_f32 = small_p.tile([1, QT], F32, name="r_f32")
            nc.vector.reciprocal_approx_fast(out=r_f32[:], in_=s_f32[:])
            r_sb = small_p.tile([1, QT], F32R, name="r_sb")
            nc.vector.tensor_copy(r_sb[:], r_f32[:])
            bc_ps = scr_psp.tile([128, 512], F32, name="scr")
            nc.tensor.matmul(
                bc_ps[0:64, :], ones128[:, 0:64], r_sb[:],
                start=True, stop=True)
            bc_sb = small_p.tile([64, QT], F32, name="bc_sb")
            nc.vector.tensor_copy(bc_sb[:], bc_ps[0:64, :])
            cn = cn_p.tile([64, QT], BF16, name="cn")
            nc.vector.tensor_mul(cn[:], ctx_ps[0:64, :], bc_sb[:])
            nc.sync.dma_start(a2a_in[hi][qt], cn[:])

        def a2a(hi):
            nc.gpsimd.collective_compute(
                "AllToAll", mybir.AluOpType.bypass,
                replica_groups=[list(range(cfg.NCORES))],
                ins=[a2a_in[hi].opt()], outs=[a2a_out[hi].opt()])

        def issue_AB(qt, fillers=()):
            """A/B-head attention q-tile; `fillers` are projection-tile
            closures sprinkled between j blocks to keep scalar fed.

            Diagonal blocks (d >= 1) trim S/exp/PV to the valid query range
            [128d, QT); the last block runs PV full-width (mask zeroes the
            stale region) so every PSUM column's accumulation group gets its
            stop flag."""
            njb = (qt + 1) * R
            ctxA = ctx_psp.tile([128, QT], F32, name="ctxA", tag="ctxA", bufs=1)
            ctxB = ctx_psp.tile([128, QT], F32, name="ctxB", tag="ctxB", bufs=1)
            nf = len(fillers)
            k = 0
            for j in range(njb):
                while k < nf and k * njb <= j * nf:
                    fillers[k]()
                    k += 1
                d = j - R * qt
                trim = 128 * d if d >= 1 else 0
                last = (j == njb - 1)
                s_ps = s_psp.tile([128, 2 * QT], F32, name="s_ps")
                for hh, (hi, po) in enumerate(((0, 0), (1, 64))):
                    nc.tensor.matmul(
                        s_ps[:, QT * hh + trim:QT * (hh + 1)],
                        k_slice(hi, j),
                        q_slice(hi, qt)[:, trim:QT],
                        start=True, stop=True, tile_position=(po, 0))
                ex = exp_sbp.tile([128, 2 * QT], BF16, name="ex")
                if trim:
                    for hh in range(2):
                        nc.scalar.activation(
                            ex[:, QT * hh + trim:QT * (hh + 1)],
                            s_ps[:, QT * hh + trim:QT * (hh + 1)],
                            EXPF, scale=scale)
                else:
                    nc.scalar.activation(ex[:], s_ps[:], EXPF, scale=scale)
                if d >= 0:
                    moff = 0 if last else trim
                    for hh in range(2):
                        nc.vector.tensor_mul(
                            ex[:, QT * hh + moff:QT * (hh + 1)],
                            ex[:, QT * hh + moff:QT * (hh + 1)],
                            masks[d][:, moff:QT])
                pvoff = 0 if last else trim
                nc.tensor.matmul(
                    ctxA[:, pvoff:QT], v_stat(0, j), ex[:, pvoff:QT],
                    start=(j == 0), stop=last)
                nc.tensor.matmul(
                    ctxB[:, pvoff:QT], v_stat(1, j),
                    ex[:, QT + pvoff:2 * QT],
                    start=(j == 0), stop=last)
            while k < nf:
                fillers[k]()
                k += 1
            norm_and_ship(0, qt, ctxA)
            norm_and_ship(1, qt, ctxB)

        def issue_C(qt):
            # njb is always even: even j blocks accumulate into ctxCe, odd
            # into ctxCo (cross-paired over both array halves); merged at ship
            njb = (qt + 1) * R
            ctxCe = ctx_psp.tile([128, QT], F32, name="ctxCe", tag="ctxA",
                                 bufs=1)
            ctxCo = ctx_psp.tile([128, QT], F32, name="ctxCo", tag="ctxB",
                                 bufs=1)
            for jg in range(0, njb, 2):
                je, jo = jg, jg + 1
                regions = []
                for i, (j, acc) in enumerate(((je, ctxCe), (jo, ctxCo))):
                    d = j - R * qt
                    st = (j == i)                    # accumulator start block
                    sp = (j == njb - 2 + i)          # accumulator stop block
                    trim = 128 * d if d >= 1 else 0
                    pvoff = 0 if (st or sp or d < 1) else trim
                    regions.append((i, j, acc, d, st, sp, trim, pvoff))
                s_ps = s_psp.tile([128, 2 * QT], F32, name="s_ps")
                for i, j, acc, d, st, sp, trim, pvoff in regions:
                    po = 64 * i
                    nc.tensor.matmul(
                        s_ps[:, QT * i + trim:QT * (i + 1)],
                        kq_ck[po:po + 64, KB * j:KB * (j + 1)],
                        kq_cq[po:po + 64,
                              QT * qt + trim:QT * (qt + 1)],
                        start=True, stop=True, tile_position=(64 * i, 0))
                ex = exp_sbp.tile([128, 2 * QT], BF16, name="ex")
                if any(r[6] for r in regions):
                    for i, j, acc, d, st, sp, trim, pvoff in regions:
                        nc.scalar.activation(
                            ex[:, QT * i + trim:QT * (i + 1)],
                            s_ps[:, QT * i + trim:QT * (i + 1)],
                            EXPF, scale=scale)
                else:
                    nc.scalar.activation(ex[:], s_ps[:], EXPF, scale=scale)
                for i, j, acc, d, st, sp, trim, pvoff in regions:
                    if d >= 0:
                        nc.vector.tensor_mul(
                            ex[:, QT * i + pvoff:QT * (i + 1)],
                            ex[:, QT * i + pvoff:QT * (i + 1)],
                            masks[d][:, pvoff:QT])
                for i, j, acc, d, st, sp, trim, pvoff in regions:
                    nc.tensor.matmul(
                        acc[:, pvoff:QT], v_stat(2, j),
                        ex[:, QT * i + pvoff:QT * (i + 1)],
                        start=st, stop=sp)
            # merge even/odd accumulators and ship
            sE = small_p.tile([1, QT], F32, name="sE")
            nc.vector.tensor_copy(sE[:], ctxCe[64:65, :])
            s_f32 = small_p.tile([1, QT], F32, name="s_f32")
            nc.vector.tensor_add(s_f32[:], ctxCo[64:65, :], sE[:])
            r_f32 = small_p.tile([1, QT], F32, name="r_f32")
            nc.vector.reciprocal_approx_fast(out=r_f32[:], in_=s_f32[:])
            r_sb = small_p.tile([1, QT], F32R, name="r_sb")
            nc.vector.tensor_copy(r_sb[:], r_f32[:])
            bc_ps = scr_psp.tile([128, 512], F32, name="scr")
            nc.tensor.matmul(
                bc_ps[0:64, :], ones128[:, 0:64], r_sb[:],
                start=True, stop=True)
            bc_sb = small_p.tile([64, QT], F32, name="bc_sb")
            nc.vector.tensor_copy(bc_sb[:], bc_ps[0:64, :])
            cE = cn_p.tile([64, QT], F32, name="cE")
            nc.vector.tensor_copy(cE[:], ctxCe[0:64, :])
            cS = cn_p.tile([64, QT], F32, name="cS")
            nc.vector.tensor_add(cS[:], ctxCo[0:64, :], cE[:])
            cn = cn_p.tile([64, QT], BF16, name="cn")
            nc.vector.tensor_mul(cn[:], cS[:], bc_sb[:])
            nc.sync.dma_start(a2a_in[2][qt], cn[:])

        # -------- pipelined issue --------
        # chunk 0 projected up front; chunk qt+1's projection tiles fill the
        # gaps inside attention q-tile qt (which only needs chunks <= qt)
        for f in a_chunk_tiles(0):
            f()
        for qt in range(N // QT):
            fillers = a_chunk_tiles(qt + 1) if qt + 1 < N // QT else ()
            issue_AB(qt, fillers)
        a2a(0)
        a2a(1)
        for qt in range(N // QT):
            issue_C(qt)
        a2a(2)

    # ---------------- Phase D: output projection ----------------
    KC_ORDER = [0, 3, 1, 2, 4, 5]   # kc needing only heads A/B first
    with (
        tc.tile_pool(name="ctxall", bufs=1) as ctxall_p,
        tc.tile_pool(name="y_ps", bufs=2, space="PSUM") as y_psp,
        tc.tile_pool(name="y_sb", bufs=3) as y_sbp,
    ):
        for bb in range(cfg.B):
            ctxall = [None] * KC
            for kc in KC_ORDER:
                ct = ctxall_p.tile([128, QT], BF16, name=f"ctxall{bb}_{kc}")
                for sub in range(2):
                    h = 2 * kc + sub
                    g, hi = h // 3, h % 3
                    nc.sync.dma_start(
                        ct[64 * sub:64 * (sub + 1), :],
                        a2a_out[hi][4 * bb + g])
                ctxall[kc] = ct
            for t in range(QT // 128):
                # lo/hi contraction halves accumulate in separate PSUM tiles
                # (single-tile-config groups), overlapped at (0,0)/(64,0);
                # merged on DVE. Bias rides in y_lo's group via ones64/bias64.
                y_lo = y_psp.tile([128, D], F32, name="y_lo")
                y_hi = y_psp.tile([128, D], F32, name="y_hi")
                tsl = slice(128 * t, 128 * (t + 1))
                for ki, kc in enumerate(KC_ORDER):
                    cl = ctxall[kc]
                    st = (ki == 0)
                    for fs, fe in ((0, 512), (512, D)):
                        nc.tensor.matmul(
                            y_lo[:, fs:fe], cl[0:64, tsl],
                            wp_sb[kc][0:64, fs:fe],
                            start=st, stop=False, tile_position=(0, 0))
                        nc.tensor.matmul(
                            y_hi[:, fs:fe], cl[64:128, tsl],
                            wp_sb[kc][64:128, fs:fe],
                            start=st, stop=(ki == KC - 1),
                            tile_position=(64, 0))
                for fs, fe in ((0, 512), (512, D)):
                    nc.tensor.matmul(
                        y_lo[:, fs:fe], ones64[:], bias64[:, fs:fe],
                        start=False, stop=True, tile_position=(0, 0))
                y_sb = y_sbp.tile([128, D], F32, name="y_sb")
                nc.vector.tensor_copy(y_sb[:], y_lo[:])
                nc.vector.tensor_add(y_sb[:], y_hi[:], y_sb[:])
                nc.sync.dma_start(
                    out_y[QT * bb + 128 * t:QT * bb + 128 * (t + 1), :],
                    y_sb[:])

    persist.release()
    dram.release()
    ctx_lp.__exit__(None, None, None)


def shard_inputs(x, W_kqv, W_proj, b_proj, cfg: Cfg):
    """Full inputs -> list of 8 per-core input dicts (numpy, host layout)."""
    HD = cfg.HD
    in_maps = []
    x = np.asarray(x, np.float32)
    W_kqv = np.asarray(W_kqv, np.float32)
    wpt = np.ascontiguousarray(
        np.asarray(W_proj, np.float32).T).astype(ml_dtypes.bfloat16)
    bias = np.ascontiguousarray(
        np.asarray(b_proj, np.float32).reshape(1, cfg.D))
    for c in range(cfg.NCORES):
        b = c // 4
        g = c % 4
        hs = [3 * g, 3 * g + 1, 3 * g + 2]
        k = [W_kqv[h][:, 0:HD] for h in hs]
        q = [W_kqv[h][:, HD:2 * HD] for h in hs]
        v = [W_kqv[h][:, 2 * HD:3 * HD] for h in hs]
        wkv = np.concatenate(
            [k[0], k[1], q[0], q[1], k[2], k[2], q[2], q[2],
             v[0], v[1], v[2]], axis=1).astype(np.float32)
        in_maps.append({
            "xT": np.ascontiguousarray(x[b].T).astype(ml_dtypes.bfloat16),
            "wkv": np.ascontiguousarray(wkv).astype(ml_dtypes.bfloat16),
            "wpt": wpt,
            "bias": bias,
        })
    return in_maps


def assemble_output(outs, cfg: Cfg):
    """Per-core y [2*QT, D] -> full [B, N, D]."""
    y = np.zeros((cfg.B, cfg.N, cfg.D), np.float32)
    for c in range(cfg.NCORES):
        o = outs[c]
        for bb in range(cfg.B):
            y[bb, cfg.QT * c:cfg.QT * (c + 1), :] = (
                o[cfg.QT * bb:cfg.QT * (bb + 1), :])
    return y


_NC_CACHE = {}


def _build_nc(cfg):
    from concourse import bacc

    nc = bacc.Bacc(
        "TRN2", target_bir_lowering=False, debug=False,
        num_devices=cfg.NCORES)
    ins = {
        "xT": nc.dram_tensor("xT", [cfg.D, cfg.N], BF16,
                             kind="ExternalInput").ap(),
        "wkv": nc.dram_tensor("wkv", [cfg.D, 704], BF16,
                              kind="ExternalInput").ap(),
        "wpt": nc.dram_tensor("wpt", [cfg.D, cfg.D], BF16,
                              kind="ExternalInput").ap(),
        "bias": nc.dram_tensor("bias", [1, cfg.D], F32,
                               kind="ExternalInput").ap(),
    }
    out = nc.dram_tensor("y", [2 * cfg.QT, cfg.D], F32,
                         kind="ExternalOutput").ap()
    with tile.TileContext(nc) as tc:
        build(tc, out, ins, cfg)
    nc.compile()
    return nc


def _get_nc(cfg):
    if "nc" not in _NC_CACHE:
        _NC_CACHE["nc"] = _build_nc(cfg)
    return _NC_CACHE["nc"]


def run_sharded(inputs, trace=False):
    import concourse.bass_utils as bass_utils

    cfg = Cfg(N=4096)
    in_maps = shard_inputs(
        inputs["x"], inputs["W_kqv"], inputs["W_proj"], inputs["b_proj"], cfg)
    nc = _get_nc(cfg)
    res = bass_utils.run_bass_kernel_spmd(
        nc, in_maps, core_ids=list(range(cfg.NCORES)), trace=trace)
    outs = [res.results[c]["y"] for c in range(cfg.NCORES)]
    return assemble_output(outs, cfg), res


def kernel(**inputs):
    y, _ = run_sharded(inputs, trace=False)
    return y


# revision 33
# speedup vs baseline: 1.0131x; 1.0131x over previous
"""Distributed causal self-attention kernel for 8 TRN2 NeuronCores (Bass/Tile).

Self-contained: kernel(**inputs) takes the FULL unsharded inputs
(x [2,4096,768], W_kqv [12,768,192], W_proj [768,768], b_proj [768]),
shards them across 8 cores (batch x head-group), runs one SPMD NEFF via
bass_utils.run_bass_kernel_spmd, and reassembles the full [2,4096,768] output.

v2: software-pipelined phases — QKV projection interleaved with A/B-head
attention per token-half so scalar-engine exp overlaps projection matmuls;
V projected directly into [keys, vdim] layout (no PE transposes); PV
stationaries are 128-col (FWL); fast approximate reciprocal; head-C pass
runs under the A/B all-to-alls so only the small head-C collective is
tail-exposed.
"""

import sys

for p in ("/opt/trn_rl_repo", "/root/.axon_site/_ro/trn_rl_repo"):
    if p not in sys.path:
        sys.path.insert(0, p)


import ml_dtypes
import numpy as np

import concourse.bass as bass
import concourse.mybir as mybir
import concourse.tile as tile

F32 = mybir.dt.float32
F32R = mybir.dt.float32r
BF16 = mybir.dt.bfloat16
EXPF = mybir.ActivationFunctionType.Exp


class Cfg:
    def __init__(self, N=4096, D=768, H=12, B=2, NCORES=8):
        self.N, self.D, self.H, self.B, self.NCORES = N, D, H, B, NCORES
        self.HD = D // H          # 64
        self.HPC = H // (NCORES // B)   # heads per core = 3
        self.KB = 128             # k block
        self.NKB = N // self.KB   # k blocks (32)
        self.QT = N // NCORES     # q tile == per-rank token chunk (512)
        self.R = self.QT // self.KB  # diag masks per q tile (4)
        self.KC = D // 128        # contraction chunks (6)
        self.VW = 65              # v block cols: 64 v dims + ones col
        self.VS = 3 * self.VW     # v_sb stride per key block (195)
        assert self.HD == 64 and self.HPC == 3 and self.QT % self.KB == 0


def r32(ap):
    return ap.bitcast(F32R) if ap.dtype == F32 else ap


def build(tc: tile.TileContext, out_y: bass.AP, ins: dict, cfg: Cfg):
    nc = tc.nc
    ctx_lp = nc.allow_low_precision(reason="fp32r matmul pipeline")
    ctx_lp.__enter__()
    N, D, QT, KB, R, KC, NKB = cfg.N, cfg.D, cfg.QT, cfg.KB, cfg.R, cfg.KC, cfg.NKB
    HD, VW, VS = cfg.HD, cfg.VW, cfg.VS
    scale = 1.0 / np.sqrt(HD)
    xT, wkv, wpt, bias = ins["xT"], ins["wkv"], ins["wpt"], ins["bias"]

    persist = tc.alloc_tile_pool(name="persist", bufs=1)
    const_p = persist

    # ---- weights + x. Few, large DMAs: the sync engine issues descriptors
    # serially (~0.6us each), so batching directly shortens startup.
    wkv_sb = []   # [weff(512) | wv(192)] per contraction chunk
    for kc in range(KC):
        w = const_p.tile([128, 704], BF16, name=f"wkv{kc}")
        nc.sync.dma_start(w[:], wkv[128 * kc:128 * (kc + 1), :])
        wkv_sb.append(w)
    xt_sb = []
    for kc in range(KC):
        x = const_p.tile([128, N], BF16, name=f"xt{kc}")
        nc.sync.dma_start(x[:, 0:1024], xT[128 * kc:128 * (kc + 1), 0:1024])
        xt_sb.append(x)
    bias_sb = const_p.tile([1, D], F32R)
    nc.sync.dma_start(bias_sb[:], bias[:].bitcast(F32R))
    # rest of x + the phase-D weights in the background
    for half in range(1, 4):
        for kc in range(KC):
            nc.sync.dma_start(
                xt_sb[kc][:, 1024 * half:1024 * (half + 1)],
                xT[128 * kc:128 * (kc + 1), 1024 * half:1024 * (half + 1)])
    wp_sb = []
    for kc in range(KC):
        w = const_p.tile([128, D], BF16, name=f"wp{kc}")
        nc.sync.dma_start(w[:], wpt[128 * kc:128 * (kc + 1), :])
        wp_sb.append(w)

    # diag masks: mask_d[p, c] = 1.0 if c >= KB*d + p else 0
    masks = []
    for d in range(R):
        mk = const_p.tile([128, QT], BF16, name=f"mask{d}")
        nc.vector.memset(mk[:], 1.0)
        nc.gpsimd.affine_select(
            out=mk[:], in_=mk[:], compare_op=mybir.AluOpType.is_ge,
            fill=0.0, base=-KB * d, pattern=[[1, QT]], channel_multiplier=-1)
        masks.append(mk)

    ones128 = const_p.tile([1, 128], F32R)
    onestage = const_p.tile([1, 128], F32)
    nc.vector.memset(onestage[:], 1.0)
    nc.vector.tensor_copy(ones128[:], onestage[:])
    # 64-partition ones / zero-padded bias pair: lets the phase-D bias add run
    # as a (64,128)-tile matmul matching the lo-half group's tile config
    ones64 = const_p.tile([64, 128], BF16)
    nc.vector.memset(ones64[:], 1.0)
    bias64 = const_p.tile([64, D], BF16)
    nc.vector.memset(bias64[:], 0.0)
    nc.vector.tensor_copy(bias64[0:1, :], bias_sb[:].bitcast(F32))

    # v in [keys, vdim] layout: per key block j cols VS*j + VW*hi + [0:64]
    # are head hi's v, col VS*j + VW*hi + 64 is the ones column. Tail pad so
    # 128-col stationary reads (FWL) stay in bounds. memset 1.0 fills the
    # ones columns; v copies overwrite the data columns.
    v_sb = const_p.tile([128, VS * NKB + 128], BF16)
    # quarter memsets so early v copies only wait on their own region
    VQ = (VS * NKB + 128) // 4
    for q in range(4):
        hi_col = VS * NKB + 128 if q == 3 else VQ * (q + 1)
        nc.vector.memset(v_sb[:, VQ * q:hi_col], 1.0)

    # persistent kq activations
    kq_ab = const_p.tile([128, 2 * N], BF16)   # p0:64 kA|qA, p64:128 kB|qB
    kq_ck = const_p.tile([128, N], BF16)       # kC duplicated in both halves
    kq_cq = const_p.tile([128, N], BF16)       # qC duplicated in both halves

    dram = tc.alloc_tile_pool(name="dram", bufs=1, space="DRAM")
    a2a_in = []
    a2a_out = []
    for hi in range(3):
        ai = dram.tile([cfg.NCORES, 64, QT], BF16, name=f"a2ain{hi}")
        ao = dram.tile([cfg.NCORES, 64, QT], BF16, name=f"a2aout{hi}")
        a2a_in.append(ai)
        a2a_out.append(ao)

    def k_slice(hi, j):
        if hi == 0:
            return kq_ab[0:64, KB * j:KB * (j + 1)]
        if hi == 1:
            return kq_ab[64:128, KB * j:KB * (j + 1)]
        return kq_ck[0:64, KB * j:KB * (j + 1)]

    def q_slice(hi, qt):
        if hi == 0:
            return kq_ab[0:64, N + QT * qt:N + QT * (qt + 1)]
        if hi == 1:
            return kq_ab[64:128, N + QT * qt:N + QT * (qt + 1)]
        return kq_cq[0:64, QT * qt:QT * (qt + 1)]

    def v_stat(hi, j):
        # 128-col stationary: cols 0:64 = v, col 64 = ones, rest junk
        return v_sb[:, VS * j + VW * hi:VS * j + VW * hi + 128]

    with (
        tc.tile_pool(name="scr_ps", bufs=2, space="PSUM") as scr_psp,
        tc.tile_pool(name="s_ps", bufs=2, space="PSUM") as s_psp,
        tc.tile_pool(name="ctx_ps", bufs=1, space="PSUM") as ctx_psp,
        tc.tile_pool(name="exp_sb", bufs=3) as exp_sbp,
        tc.tile_pool(name="small_sb", bufs=3) as small_p,
        tc.tile_pool(name="cn_sb", bufs=3) as cn_p,
    ):
        def kq_dst(mt, gch):
            fr = slice(512 * gch, 512 * (gch + 1))
            if mt == 0:
                return kq_ab[:, fr]
            if mt == 1:
                return kq_ab[:, N + 512 * gch:N + 512 * (gch + 1)]
            if mt == 2:
                return kq_ck[:, fr]
            return kq_cq[:, fr]

        def kq_tile(mt, gch):
            fr = slice(512 * gch, 512 * (gch + 1))
            ps = scr_psp.tile([128, 512], F32, name="scr")
            for kc in range(KC):
                nc.tensor.matmul(
                    ps[:], wkv_sb[kc][:, 128 * mt:128 * (mt + 1)],
                    xt_sb[kc][:, fr],
                    start=(kc == 0), stop=(kc == KC - 1))
            nc.vector.tensor_copy(kq_dst(mt, gch), ps[:])

        def v_tile(j):
            tfr = slice(128 * j, 128 * (j + 1))
            ps = scr_psp.tile([128, 512], F32, name="scr")
            for kc in range(KC):
                nc.tensor.matmul(
                    ps[:, 0:192], xt_sb[kc][:, tfr], wkv_sb[kc][:, 512:704],
                    start=(kc == 0), stop=(kc == KC - 1))
            for hi in range(3):
                nc.vector.tensor_copy(
                    v_sb[:, VS * j + VW * hi:VS * j + VW * hi + 64],
                    ps[:, 64 * hi:64 * (hi + 1)])

        def a_chunk_tiles(gch):
            """Projection work for one 512-token chunk as 8 filler closures."""
            fillers = [
                (lambda mt=mt: kq_tile(mt, gch)) for mt in range(4)]
            fillers += [
                (lambda j=j: v_tile(j)) for j in range(4 * gch, 4 * gch + 4)]
            return fillers

        def norm_and_ship(hi, qt, ctx_ps):
            # custom DVE ops misread partition-offset inputs: stage the sums
            # row at partition 0 before the approx reciprocal
            s_f32 = small_p.tile([1, QT], F32, name="s_f32")
            nc.vector.tensor_copy(s_f32[:], ctx_ps[64:65, :])
            r_f32 = small_p.tile([1, QT], F32, name="r_f32")
            nc.vector.reciprocal_approx_fast(out=r_f32[:], in_=s_f32[:])
            r_sb = small_p.tile([1, QT], F32R, name="r_sb")
            nc.vector.tensor_copy(r_sb[:], r_f32[:])
            bc_ps = scr_psp.tile([128, 512], F32, name="scr")
            nc.tensor.matmul(
                bc_ps[0:64, :], ones128[:, 0:64], r_sb[:],
                start=True, stop=True)
            bc_sb = small_p.tile([64, QT], F32, name="bc_sb")
            nc.vector.tensor_copy(bc_sb[:], bc_ps[0:64, :])
            cn = cn_p.tile([64, QT], BF16, name="cn")
            nc.vector.tensor_mul(cn[:], ctx_ps[0:64, :], bc_sb[:])
            nc.sync.dma_start(a2a_in[hi][qt], cn[:])

        def a2a(hi):
            nc.gpsimd.collective_compute(
                "AllToAll", mybir.AluOpType.bypass,
                replica_groups=[list(range(cfg.NCORES))],
                ins=[a2a_in[hi].opt()], outs=[a2a_out[hi].opt()])

        def issue_AB(qt, fillers=()):
            """A/B-head attention q-tile; `fillers` are projection-tile
            closures sprinkled between j blocks to keep scalar fed.

            Diagonal blocks (d >= 1) trim S/exp/PV to the valid query range
            [128d, QT); the last block runs PV full-width (mask zeroes the
            stale region) so every PSUM column's accumulation group gets its
            stop flag."""
            njb = (qt + 1) * R
            ctxA = ctx_psp.tile([128, QT], F32, name="ctxA", tag="ctxA", bufs=1)
            ctxB = ctx_psp.tile([128, QT], F32, name="ctxB", tag="ctxB", bufs=1)
            nf = len(fillers)
            k = 0
            for j in range(njb):
                while k < nf and k * njb <= j * nf:
                    fillers[k]()
                    k += 1
                d = j - R * qt
                trim = 128 * d if d >= 1 else 0
                last = (j == njb - 1)
                s_ps = s_psp.tile([128, 2 * QT], F32, name="s_ps")
                for hh, (hi, po) in enumerate(((0, 0), (1, 64))):
                    nc.tensor.matmul(
                        s_ps[:, QT * hh + trim:QT * (hh + 1)],
                        k_slice(hi, j),
                        q_slice(hi, qt)[:, trim:QT],
                        start=True, stop=True, tile_position=(po, 0))
                ex = exp_sbp.tile([128, 2 * QT], BF16, name="ex")
                if trim:
                    for hh in range(2):
                        nc.scalar.activation(
                            ex[:, QT * hh + trim:QT * (hh + 1)],
                            s_ps[:, QT * hh + trim:QT * (hh + 1)],
                            EXPF, scale=scale)
                else:
                    nc.scalar.activation(ex[:], s_ps[:], EXPF, scale=scale)
                if d >= 0:
                    moff = 0 if last else trim
                    for hh in range(2):
                        nc.vector.tensor_mul(
                            ex[:, QT * hh + moff:QT * (hh + 1)],
                            ex[:, QT * hh + moff:QT * (hh + 1)],
                            masks[d][:, moff:QT])
                pvoff = 0 if last else trim
                nc.tensor.matmul(
                    ctxA[:, pvoff:QT], v_stat(0, j), ex[:, pvoff:QT],
                    start=(j == 0), stop=last)
                nc.tensor.matmul(
                    ctxB[:, pvoff:QT], v_stat(1, j),
                    ex[:, QT + pvoff:2 * QT],
                    start=(j == 0), stop=last)
            while k < nf:
                fillers[k]()
                k += 1
            norm_and_ship(0, qt, ctxA)
            norm_and_ship(1, qt, ctxB)

        def issue_C(qt):
            # njb is always even: even j blocks accumulate into ctxCe, odd
            # into ctxCo (cross-paired over both array halves); merged at ship
            njb = (qt + 1) * R
            ctxCe = ctx_psp.tile([128, QT], F32, name="ctxCe", tag="ctxA",
                                 bufs=1)
            ctxCo = ctx_psp.tile([128, QT], F32, name="ctxCo", tag="ctxB",
                                 bufs=1)
            for jg in range(0, njb, 2):
                je, jo = jg, jg + 1
                regions = []
                for i, (j, acc) in enumerate(((je, ctxCe), (jo, ctxCo))):
                    d = j - R * qt
                    st = (j == i)                    # accumulator start block
                    sp = (j == njb - 2 + i)          # accumulator stop block
                    trim = 128 * d if d >= 1 else 0
                    pvoff = 0 if (st or sp or d < 1) else trim
                    regions.append((i, j, acc, d, st, sp, trim, pvoff))
                s_ps = s_psp.tile([128, 2 * QT], F32, name="s_ps")
                for i, j, acc, d, st, sp, trim, pvoff in regions:
                    po = 64 * i
                    nc.tensor.matmul(
                        s_ps[:, QT * i + trim:QT * (i + 1)],
                        kq_ck[po:po + 64, KB * j:KB * (j + 1)],
                        kq_cq[po:po + 64,
                              QT * qt + trim:QT * (qt + 1)],
                        start=True, stop=True, tile_position=(64 * i, 0))
                ex = exp_sbp.tile([128, 2 * QT], BF16, name="ex")
                if any(r[6] for r in regions):
                    for i, j, acc, d, st, sp, trim, pvoff in regions:
                        nc.scalar.activation(
                            ex[:, QT * i + trim:QT * (i + 1)],
                            s_ps[:, QT * i + trim:QT * (i + 1)],
                            EXPF, scale=scale)
                else:
                    nc.scalar.activation(ex[:], s_ps[:], EXPF, scale=scale)
                for i, j, acc, d, st, sp, trim, pvoff in regions:
                    if d >= 0:
                        nc.vector.tensor_mul(
                            ex[:, QT * i + pvoff:QT * (i + 1)],
                            ex[:, QT * i + pvoff:QT * (i + 1)],
                            masks[d][:, pvoff:QT])
                for i, j, acc, d, st, sp, trim, pvoff in regions:
                    nc.tensor.matmul(
                        acc[:, pvoff:QT], v_stat(2, j),
                        ex[:, QT * i + pvoff:QT * (i + 1)],
                        start=st, stop=sp)
            # merge even/odd accumulators and ship
            sE = small_p.tile([1, QT], F32, name="sE")
            nc.vector.tensor_copy(sE[:], ctxCe[64:65, :])
            s_f32 = small_p.tile([1, QT], F32, name="s_f32")
            nc.vector.tensor_add(s_f32[:], ctxCo[64:65, :], sE[:])
            r_f32 = small_p.tile([1, QT], F32, name="r_f32")
            nc.vector.reciprocal_approx_fast(out=r_f32[:], in_=s_f32[:])
            r_sb = small_p.tile([1, QT], F32R, name="r_sb")
            nc.vector.tensor_copy(r_sb[:], r_f32[:])
            bc_ps = scr_psp.tile([128, 512], F32, name="scr")
            nc.tensor.matmul(
                bc_ps[0:64, :], ones128[:, 0:64], r_sb[:],
                start=True, stop=True)
            bc_sb = small_p.tile([64, QT], F32, name="bc_sb")
            nc.vector.tensor_copy(bc_sb[:], bc_ps[0:64, :])
            cE = cn_p.tile([64, QT], F32, name="cE")
            nc.vector.tensor_copy(cE[:], ctxCe[0:64, :])
            cS = cn_p.tile([64, QT], F32, name="cS")
            nc.vector.tensor_add(cS[:], ctxCo[0:64, :], cE[:])
            cn = cn_p.tile([64, QT], BF16, name="cn")
            nc.vector.tensor_mul(cn[:], cS[:], bc_sb[:])
            nc.sync.dma_start(a2a_in[2][qt], cn[:])

        # -------- pipelined issue --------
        # chunk 0 projected up front; chunk qt+1's projection tiles fill the
        # gaps inside attention q-tile qt (which only needs chunks <= qt)
        for f in a_chunk_tiles(0):
            f()
        for qt in range(N // QT):
            fillers = a_chunk_tiles(qt + 1) if qt + 1 < N // QT else ()
            issue_AB(qt, fillers)
        a2a(0)
        a2a(1)
        for qt in range(N // QT):
            issue_C(qt)
        a2a(2)

    # ---------------- Phase D: output projection ----------------
    KC_ORDER = [0, 3, 1, 2, 4, 5]   # kc needing only heads A/B first
    with (
        tc.tile_pool(name="ctxall", bufs=1) as ctxall_p,
        tc.tile_pool(name="y_ps", bufs=2, space="PSUM") as y_psp,
        tc.tile_pool(name="y_sb", bufs=3) as y_sbp,
    ):
        for bb in range(cfg.B):
            ctxall = [None] * KC
            for kc in KC_ORDER:
                ct = ctxall_p.tile([128, QT], BF16, name=f"ctxall{bb}_{kc}")
                for sub in range(2):
                    h = 2 * kc + sub
                    g, hi = h // 3, h % 3
                    nc.sync.dma_start(
                        ct[64 * sub:64 * (sub + 1), :],
                        a2a_out[hi][4 * bb + g])
                ctxall[kc] = ct
            for t in range(QT // 128):
                # lo/hi contraction halves accumulate in separate PSUM tiles
                # (single-tile-config groups), overlapped at (0,0)/(64,0);
                # merged on DVE. Bias rides in y_lo's group via ones64/bias64.
                y_lo = y_psp.tile([128, D], F32, name="y_lo")
                y_hi = y_psp.tile([128, D], F32, name="y_hi")
                tsl = slice(128 * t, 128 * (t + 1))
                for ki, kc in enumerate(KC_ORDER):
                    cl = ctxall[kc]
                    st = (ki == 0)
                    for fs, fe in ((0, 512), (512, D)):
                        nc.tensor.matmul(
                            y_lo[:, fs:fe], cl[0:64, tsl],
                            wp_sb[kc][0:64, fs:fe],
                            start=st, stop=False, tile_position=(0, 0))
                        nc.tensor.matmul(
                            y_hi[:, fs:fe], cl[64:128, tsl],
                            wp_sb[kc][64:128, fs:fe],
                            start=st, stop=(ki == KC - 1),
                            tile_position=(64, 0))
                for fs, fe in ((0, 512), (512, D)):
                    nc.tensor.matmul(
                        y_lo[:, fs:fe], ones64[:], bias64[:, fs:fe],
                        start=False, stop=True, tile_position=(0, 0))
                y_sb = y_sbp.tile([128, D], F32, name="y_sb")
                nc.vector.tensor_copy(y_sb[:], y_lo[:])
                nc.vector.tensor_add(y_sb[:], y_hi[:], y_sb[:])
                nc.sync.dma_start(
                    out_y[QT * bb + 128 * t:QT * bb + 128 * (t + 1), :],
                    y_sb[:])

    persist.release()
    dram.release()
    ctx_lp.__exit__(None, None, None)


def shard_inputs(x, W_kqv, W_proj, b_proj, cfg: Cfg):
    """Full inputs -> list of 8 per-core input dicts (numpy, host layout)."""
    HD = cfg.HD
    in_maps = []
    x = np.asarray(x, np.float32)
    W_kqv = np.asarray(W_kqv, np.float32)
    wpt = np.ascontiguousarray(
        np.asarray(W_proj, np.float32).T).astype(ml_dtypes.bfloat16)
    bias = np.ascontiguousarray(
        np.asarray(b_proj, np.float32).reshape(1, cfg.D))
    for c in range(cfg.NCORES):
        b = c // 4
        g = c % 4
        hs = [3 * g, 3 * g + 1, 3 * g + 2]
        k = [W_kqv[h][:, 0:HD] for h in hs]
        q = [W_kqv[h][:, HD:2 * HD] for h in hs]
        v = [W_kqv[h][:, 2 * HD:3 * HD] for h in hs]
        wkv = np.concatenate(
            [k[0], k[1], q[0], q[1], k[2], k[2], q[2], q[2],
             v[0], v[1], v[2]], axis=1).astype(np.float32)
        in_maps.append({
            "xT": np.ascontiguousarray(x[b].T).astype(ml_dtypes.bfloat16),
            "wkv": np.ascontiguousarray(wkv).astype(ml_dtypes.bfloat16),
            "wpt": wpt,
            "bias": bias,
        })
    return in_maps


def assemble_output(outs, cfg: Cfg):
    """Per-core y [2*QT, D] -> full [B, N, D]."""
    y = np.zeros((cfg.B, cfg.N, cfg.D), np.float32)
    for c in range(cfg.NCORES):
        o = outs[c]
        for bb in range(cfg.B):
            y[bb, cfg.QT * c:cfg.QT * (c + 1), :] = (
                o[cfg.QT * bb:cfg.QT * (bb + 1), :])
    return y


_NC_CACHE = {}


def _build_nc(cfg):
    from concourse import bacc

    nc = bacc.Bacc(
        "TRN2", target_bir_lowering=False, debug=False,
        num_devices=cfg.NCORES)
    ins = {
        "xT": nc.dram_tensor("xT", [cfg.D, cfg.N], BF16,
                             kind="ExternalInput").ap(),
        "wkv": nc.dram_tensor("wkv", [cfg.D, 704], BF16,
                              kind="ExternalInput").ap(),
        "wpt": nc.dram_tensor("wpt", [cfg.D, cfg.D], BF16,
                              kind="ExternalInput").ap(),
        "bias": nc.dram_tensor("bias", [1, cfg.D], F32,
                               kind="ExternalInput").ap(),
    }
    out = nc.dram_tensor("y", [2 * cfg.QT, cfg.D], F32,
                         kind="ExternalOutput").ap()
    with tile.TileContext(nc) as tc:
        build(tc, out, ins, cfg)
    nc.compile()
    return nc


def _get_nc(cfg):
    if "nc" not in _NC_CACHE:
        _NC_CACHE["nc"] = _build_nc(cfg)
    return _NC_CACHE["nc"]


def run_sharded(inputs, trace=False):
    import concourse.bass_utils as bass_utils

    cfg = Cfg(N=4096)
    in_maps = shard_inputs(
        inputs["x"], inputs["W_kqv"], inputs["W_proj"], inputs["b_proj"], cfg)
    nc = _get_nc(cfg)
    res = bass_utils.run_bass_kernel_spmd(
        nc, in_maps, core_ids=list(range(cfg.NCORES)), trace=trace)
    outs = [res.results[c]["y"] for c in range(cfg.NCORES)]
    return assemble_output(outs, cfg), res


def kernel(**inputs):
    y, _ = run_sharded(inputs, trace=False)
    return y


# revision 39
# speedup vs baseline: 1.0167x; 1.0035x over previous
"""Distributed causal self-attention kernel for 8 TRN2 NeuronCores (Bass/Tile).

Self-contained: kernel(**inputs) takes the FULL unsharded inputs
(x [2,4096,768], W_kqv [12,768,192], W_proj [768,768], b_proj [768]),
shards them across 8 cores (batch x head-group), runs one SPMD NEFF via
bass_utils.run_bass_kernel_spmd, and reassembles the full [2,4096,768] output.

v2: software-pipelined phases — QKV projection interleaved with A/B-head
attention per token-half so scalar-engine exp overlaps projection matmuls;
V projected directly into [keys, vdim] layout (no PE transposes); PV
stationaries are 128-col (FWL); fast approximate reciprocal; head-C pass
runs under the A/B all-to-alls so only the small head-C collective is
tail-exposed.
"""

import sys

for p in ("/opt/trn_rl_repo", "/root/.axon_site/_ro/trn_rl_repo"):
    if p not in sys.path:
        sys.path.insert(0, p)


import ml_dtypes
import numpy as np

import concourse.bass as bass
import concourse.mybir as mybir
import concourse.tile as tile

F32 = mybir.dt.float32
F32R = mybir.dt.float32r
BF16 = mybir.dt.bfloat16
EXPF = mybir.ActivationFunctionType.Exp


class Cfg:
    def __init__(self, N=4096, D=768, H=12, B=2, NCORES=8):
        self.N, self.D, self.H, self.B, self.NCORES = N, D, H, B, NCORES
        self.HD = D // H          # 64
        self.HPC = H // (NCORES // B)   # heads per core = 3
        self.KB = 128             # k block
        self.NKB = N // self.KB   # k blocks (32)
        self.QT = N // NCORES     # q tile == per-rank token chunk (512)
        self.R = self.QT // self.KB  # diag masks per q tile (4)
        self.KC = D // 128        # contraction chunks (6)
        self.VW = 65              # v block cols: 64 v dims + ones col
        self.VS = 3 * self.VW     # v_sb stride per key block (195)
        assert self.HD == 64 and self.HPC == 3 and self.QT % self.KB == 0


def r32(ap):
    return ap.bitcast(F32R) if ap.dtype == F32 else ap


def build(tc: tile.TileContext, out_y: bass.AP, ins: dict, cfg: Cfg):
    nc = tc.nc
    ctx_lp = nc.allow_low_precision(reason="fp32r matmul pipeline")
    ctx_lp.__enter__()
    N, D, QT, KB, R, KC, NKB = cfg.N, cfg.D, cfg.QT, cfg.KB, cfg.R, cfg.KC, cfg.NKB
    HD, VW, VS = cfg.HD, cfg.VW, cfg.VS
    scale = 1.0 / np.sqrt(HD)
    xT, wkv, wpt, bias = ins["xT"], ins["wkv"], ins["wpt"], ins["bias"]

    persist = tc.alloc_tile_pool(name="persist", bufs=1)
    const_p = persist

    # ---- weights + x. Few, large DMAs: the sync engine issues descriptors
    # serially (~0.6us each), so batching directly shortens startup.
    wkv_sb = []   # [kA|kB | qA|qB | kC|qC | vA|vB|vC] per contraction chunk
    for kc in range(KC):
        w = const_p.tile([128, 576], BF16, name=f"wkv{kc}")
        nc.sync.dma_start(w[:], wkv[128 * kc:128 * (kc + 1), :])
        wkv_sb.append(w)
    xt_sb = []
    for kc in range(KC):
        x = const_p.tile([128, N], BF16, name=f"xt{kc}")
        nc.sync.dma_start(x[:, 0:1024], xT[128 * kc:128 * (kc + 1), 0:1024])
        xt_sb.append(x)
    bias_sb = const_p.tile([1, D], F32R)
    nc.sync.dma_start(bias_sb[:], bias[:].bitcast(F32R))
    # rest of x + the phase-D weights in the background
    for half in range(1, 4):
        for kc in range(KC):
            nc.sync.dma_start(
                xt_sb[kc][:, 1024 * half:1024 * (half + 1)],
                xT[128 * kc:128 * (kc + 1), 1024 * half:1024 * (half + 1)])
    wp_sb = []
    for kc in range(KC):
        w = const_p.tile([128, D], BF16, name=f"wp{kc}")
        nc.sync.dma_start(w[:], wpt[128 * kc:128 * (kc + 1), :])
        wp_sb.append(w)

    # diag masks: mask_d[p, c] = 1.0 if c >= KB*d + p else 0
    masks = []
    for d in range(R):
        mk = const_p.tile([128, QT], BF16, name=f"mask{d}")
        nc.vector.memset(mk[:], 1.0)
        nc.gpsimd.affine_select(
            out=mk[:], in_=mk[:], compare_op=mybir.AluOpType.is_ge,
            fill=0.0, base=-KB * d, pattern=[[1, QT]], channel_multiplier=-1)
        masks.append(mk)

    ones128 = const_p.tile([1, 128], F32R)
    onestage = const_p.tile([1, 128], F32)
    nc.vector.memset(onestage[:], 1.0)
    nc.vector.tensor_copy(ones128[:], onestage[:])
    # 64-partition ones / zero-padded bias pair: lets the phase-D bias add run
    # as a (64,128)-tile matmul matching the lo-half group's tile config
    ones64 = const_p.tile([64, 128], BF16)
    nc.vector.memset(ones64[:], 1.0)
    bias64 = const_p.tile([64, D], BF16)
    nc.vector.memset(bias64[:], 0.0)
    nc.vector.tensor_copy(bias64[0:1, :], bias_sb[:].bitcast(F32))

    # v in [keys, vdim] layout: per key block j cols VS*j + VW*hi + [0:64]
    # are head hi's v, col VS*j + VW*hi + 64 is the ones column. Tail pad so
    # 128-col stationary reads (FWL) stay in bounds. memset 1.0 fills the
    # ones columns; v copies overwrite the data columns.
    v_sb = const_p.tile([128, VS * NKB + 128], BF16)
    # quarter memsets so early v copies only wait on their own region
    VQ = (VS * NKB + 128) // 4
    for q in range(4):
        hi_col = VS * NKB + 128 if q == 3 else VQ * (q + 1)
        nc.vector.memset(v_sb[:, VQ * q:hi_col], 1.0)

    # persistent kq activations
    kq_ab = const_p.tile([128, 2 * N], BF16)   # p0:64 kA|qA, p64:128 kB|qB
    kq_ck = const_p.tile([128, N], BF16)       # kC duplicated in both halves
    kq_cq = const_p.tile([128, N], BF16)       # qC duplicated in both halves

    dram = tc.alloc_tile_pool(name="dram", bufs=1, space="DRAM")
    a2a_in = []
    a2a_out = []
    for hi in range(3):
        ai = dram.tile([cfg.NCORES, 64, QT], BF16, name=f"a2ain{hi}")
        ao = dram.tile([cfg.NCORES, 64, QT], BF16, name=f"a2aout{hi}")
        a2a_in.append(ai)
        a2a_out.append(ao)

    def k_slice(hi, j):
        if hi == 0:
            return kq_ab[0:64, KB * j:KB * (j + 1)]
        if hi == 1:
            return kq_ab[64:128, KB * j:KB * (j + 1)]
        return kq_ck[0:64, KB * j:KB * (j + 1)]

    def q_slice(hi, qt):
        if hi == 0:
            return kq_ab[0:64, N + QT * qt:N + QT * (qt + 1)]
        if hi == 1:
            return kq_ab[64:128, N + QT * qt:N + QT * (qt + 1)]
        return kq_cq[0:64, QT * qt:QT * (qt + 1)]

    def v_stat(hi, j):
        # 128-col stationary: cols 0:64 = v, col 64 = ones, rest junk
        return v_sb[:, VS * j + VW * hi:VS * j + VW * hi + 128]

    with (
        tc.tile_pool(name="scr_ps", bufs=2, space="PSUM") as scr_psp,
        tc.tile_pool(name="s_ps", bufs=2, space="PSUM") as s_psp,
        tc.tile_pool(name="ctx_ps", bufs=1, space="PSUM") as ctx_psp,
        tc.tile_pool(name="exp_sb", bufs=3) as exp_sbp,
        tc.tile_pool(name="small_sb", bufs=3) as small_p,
        tc.tile_pool(name="cn_sb", bufs=3) as cn_p,
    ):
        def kq_tile(mt, gch):
            fr = slice(512 * gch, 512 * (gch + 1))
            ps = scr_psp.tile([128, 512], F32, name="scr")
            for kc in range(KC):
                nc.tensor.matmul(
                    ps[:], wkv_sb[kc][:, 128 * mt:128 * (mt + 1)],
                    xt_sb[kc][:, fr],
                    start=(kc == 0), stop=(kc == KC - 1))
            if mt == 0:
                nc.vector.tensor_copy(kq_ab[:, fr], ps[:])
            elif mt == 1:
                nc.vector.tensor_copy(
                    kq_ab[:, N + 512 * gch:N + 512 * (gch + 1)], ps[:])
            else:
                # mt 2 = [kC | qC] computed once; duplicate into both
                # partition halves (32-partition copies route freely
                # across quadrants)
                nc.vector.tensor_copy(kq_ck[0:64, fr], ps[0:64, :])
                nc.vector.tensor_copy(kq_ck[64:96, fr], ps[0:32, :])
                nc.vector.tensor_copy(kq_ck[96:128, fr], ps[32:64, :])
                nc.vector.tensor_copy(kq_cq[64:128, fr], ps[64:128, :])
                nc.vector.tensor_copy(kq_cq[0:32, fr], ps[64:96, :])
                nc.vector.tensor_copy(kq_cq[32:64, fr], ps[96:128, :])

        def v_tile(j):
            tfr = slice(128 * j, 128 * (j + 1))
            ps = scr_psp.tile([128, 512], F32, name="scr")
            for kc in range(KC):
                nc.tensor.matmul(
                    ps[:, 0:192], xt_sb[kc][:, tfr], wkv_sb[kc][:, 384:576],
                    start=(kc == 0), stop=(kc == KC - 1))
            for hi in range(3):
                nc.vector.tensor_copy(
                    v_sb[:, VS * j + VW * hi:VS * j + VW * hi + 64],
                    ps[:, 64 * hi:64 * (hi + 1)])

        def a_chunk_tiles(gch):
            """Projection work for one 512-token chunk as 8 filler closures."""
            fillers = [
                (lambda mt=mt: kq_tile(mt, gch)) for mt in range(3)]
            fillers += [
                (lambda j=j: v_tile(j)) for j in range(4 * gch, 4 * gch + 4)]
            return fillers

        def norm_and_ship(hi, qt, ctx_ps):
            # custom DVE ops misread partition-offset inputs: stage the sums
            # row at partition 0 before the approx reciprocal
            s_f32 = small_p.tile([1, QT], F32, name="s_f32")
            nc.vector.tensor_copy(s_f32[:], ctx_ps[64:65, :])
            r_f32 = small_p.tile([1, QT], F32, name="r_f32")
            nc.vector.reciprocal_approx_fast(out=r_f32[:], in_=s_f32[:])
            r_sb = small_p.tile([1, QT], F32R, name="r_sb")
            nc.vector.tensor_copy(r_sb[:], r_f32[:])
            bc_ps = scr_psp.tile([128, 512], F32, name="scr")
            nc.tensor.matmul(
                bc_ps[0:64, :], ones128[:, 0:64], r_sb[:],
                start=True, stop=True)
            bc_sb = small_p.tile([64, QT], F32, name="bc_sb")
            nc.vector.tensor_copy(bc_sb[:], bc_ps[0:64, :])
            cn = cn_p.tile([64, QT], BF16, name="cn")
            nc.vector.tensor_mul(cn[:], ctx_ps[0:64, :], bc_sb[:])
            nc.sync.dma_start(a2a_in[hi][qt], cn[:])

        def a2a(hi):
            nc.gpsimd.collective_compute(
                "AllToAll", mybir.AluOpType.bypass,
                replica_groups=[list(range(cfg.NCORES))],
                ins=[a2a_in[hi].opt()], outs=[a2a_out[hi].opt()])

        def issue_AB(qt, fillers=()):
            """A/B-head attention q-tile; `fillers` are projection-tile
            closures sprinkled between j blocks to keep scalar fed.

            Diagonal blocks (d >= 1) trim S/exp/PV to the valid query range
            [128d, QT); the last block runs PV full-width (mask zeroes the
            stale region) so every PSUM column's accumulation group gets its
            stop flag."""
            njb = (qt + 1) * R
            ctxA = ctx_psp.tile([128, QT], F32, name="ctxA", tag="ctxA", bufs=1)
            ctxB = ctx_psp.tile([128, QT], F32, name="ctxB", tag="ctxB", bufs=1)
            nf = len(fillers)
            k = 0
            for j in range(njb):
                while k < nf and k * njb <= j * nf:
                    fillers[k]()
                    k += 1
                d = j - R * qt
                trim = 128 * d if d >= 1 else 0
                last = (j == njb - 1)
                s_ps = s_psp.tile([128, 2 * QT], F32, name="s_ps")
                for hh, (hi, po) in enumerate(((0, 0), (1, 64))):
                    nc.tensor.matmul(
                        s_ps[:, QT * hh + trim:QT * (hh + 1)],
                        k_slice(hi, j),
                        q_slice(hi, qt)[:, trim:QT],
                        start=True, stop=True, tile_position=(po, 0))
                ex = exp_sbp.tile([128, 2 * QT], BF16, name="ex")
                if trim:
                    for hh in range(2):
                        nc.scalar.activation(
                            ex[:, QT * hh + trim:QT * (hh + 1)],
                            s_ps[:, QT * hh + trim:QT * (hh + 1)],
                            EXPF, scale=scale)
                else:
                    nc.scalar.activation(ex[:], s_ps[:], EXPF, scale=scale)
                if d >= 0:
                    moff = 0 if last else trim
                    for hh in range(2):
                        nc.vector.tensor_mul(
                            ex[:, QT * hh + moff:QT * (hh + 1)],
                            ex[:, QT * hh + moff:QT * (hh + 1)],
                            masks[d][:, moff:QT])
                pvoff = 0 if last else trim
                nc.tensor.matmul(
                    ctxA[:, pvoff:QT], v_stat(0, j), ex[:, pvoff:QT],
                    start=(j == 0), stop=last)
                nc.tensor.matmul(
                    ctxB[:, pvoff:QT], v_stat(1, j),
                    ex[:, QT + pvoff:2 * QT],
                    start=(j == 0), stop=last)
            while k < nf:
                fillers[k]()
                k += 1
            norm_and_ship(0, qt, ctxA)
            norm_and_ship(1, qt, ctxB)

        def issue_C(qt):
            # njb is always even: even j blocks accumulate into ctxCe, odd
            # into ctxCo (cross-paired over both array halves); merged at ship
            njb = (qt + 1) * R
            ctxCe = ctx_psp.tile([128, QT], F32, name="ctxCe", tag="ctxA",
                                 bufs=1)
            ctxCo = ctx_psp.tile([128, QT], F32, name="ctxCo", tag="ctxB",
                                 bufs=1)
            for jg in range(0, njb, 2):
                je, jo = jg, jg + 1
                regions = []
                for i, (j, acc) in enumerate(((je, ctxCe), (jo, ctxCo))):
                    d = j - R * qt
                    st = (j == i)                    # accumulator start block
                    sp = (j == njb - 2 + i)          # accumulator stop block
                    trim = 128 * d if d >= 1 else 0
                    pvoff = 0 if (st or sp or d < 1) else trim
                    regions.append((i, j, acc, d, st, sp, trim, pvoff))
                s_ps = s_psp.tile([128, 2 * QT], F32, name="s_ps")
                for i, j, acc, d, st, sp, trim, pvoff in regions:
                    po = 64 * i
                    nc.tensor.matmul(
                        s_ps[:, QT * i + trim:QT * (i + 1)],
                        kq_ck[po:po + 64, KB * j:KB * (j + 1)],
                        kq_cq[po:po + 64,
                              QT * qt + trim:QT * (qt + 1)],
                        start=True, stop=True, tile_position=(64 * i, 0))
                ex = exp_sbp.tile([128, 2 * QT], BF16, name="ex")
                if any(r[6] for r in regions):
                    for i, j, acc, d, st, sp, trim, pvoff in regions:
                        nc.scalar.activation(
                            ex[:, QT * i + trim:QT * (i + 1)],
                            s_ps[:, QT * i + trim:QT * (i + 1)],
                            EXPF, scale=scale)
                else:
                    nc.scalar.activation(ex[:], s_ps[:], EXPF, scale=scale)
                for i, j, acc, d, st, sp, trim, pvoff in regions:
                    if d >= 0:
                        nc.vector.tensor_mul(
                            ex[:, QT * i + pvoff:QT * (i + 1)],
                            ex[:, QT * i + pvoff:QT * (i + 1)],
                            masks[d][:, pvoff:QT])
                for i, j, acc, d, st, sp, trim, pvoff in regions:
                    nc.tensor.matmul(
                        acc[:, pvoff:QT], v_stat(2, j),
                        ex[:, QT * i + pvoff:QT * (i + 1)],
                        start=st, stop=sp)
            # merge even/odd accumulators and ship
            sE = small_p.tile([1, QT], F32, name="sE")
            nc.vector.tensor_copy(sE[:], ctxCe[64:65, :])
            s_f32 = small_p.tile([1, QT], F32, name="s_f32")
            nc.vector.tensor_add(s_f32[:], ctxCo[64:65, :], sE[:])
            r_f32 = small_p.tile([1, QT], F32, name="r_f32")
            nc.vector.reciprocal_approx_fast(out=r_f32[:], in_=s_f32[:])
            r_sb = small_p.tile([1, QT], F32R, name="r_sb")
            nc.vector.tensor_copy(r_sb[:], r_f32[:])
            bc_ps = scr_psp.tile([128, 512], F32, name="scr")
            nc.tensor.matmul(
                bc_ps[0:64, :], ones128[:, 0:64], r_sb[:],
                start=True, stop=True)
            bc_sb = small_p.tile([64, QT], F32, name="bc_sb")
            nc.vector.tensor_copy(bc_sb[:], bc_ps[0:64, :])
            cE = cn_p.tile([64, QT], F32, name="cE")
            nc.vector.tensor_copy(cE[:], ctxCe[0:64, :])
            cS = cn_p.tile([64, QT], F32, name="cS")
            nc.vector.tensor_add(cS[:], ctxCo[0:64, :], cE[:])
            cn = cn_p.tile([64, QT], BF16, name="cn")
            nc.vector.tensor_mul(cn[:], cS[:], bc_sb[:])
            nc.sync.dma_start(a2a_in[2][qt], cn[:])

        # -------- pipelined issue --------
        # chunk 0 projected up front; chunk qt+1's projection tiles fill the
        # gaps inside attention q-tile qt (which only needs chunks <= qt)
        for f in a_chunk_tiles(0):
            f()
        for qt in range(N // QT):
            fillers = a_chunk_tiles(qt + 1) if qt + 1 < N // QT else ()
            issue_AB(qt, fillers)
        a2a(0)
        a2a(1)
        for qt in range(N // QT):
            issue_C(qt)
        a2a(2)

    # ---------------- Phase D: output projection ----------------
    KC_ORDER = [0, 3, 1, 2, 4, 5]   # kc needing only heads A/B first
    with (
        tc.tile_pool(name="ctxall", bufs=1) as ctxall_p,
        tc.tile_pool(name="y_ps", bufs=2, space="PSUM") as y_psp,
        tc.tile_pool(name="y_sb", bufs=3) as y_sbp,
    ):
        for bb in range(cfg.B):
            ctxall = [None] * KC
            for kc in KC_ORDER:
                ct = ctxall_p.tile([128, QT], BF16, name=f"ctxall{bb}_{kc}")
                for sub in range(2):
                    h = 2 * kc + sub
                    g, hi = h // 3, h % 3
                    nc.sync.dma_start(
                        ct[64 * sub:64 * (sub + 1), :],
                        a2a_out[hi][4 * bb + g])
                ctxall[kc] = ct
            for t in range(QT // 128):
                # lo/hi contraction halves accumulate in separate PSUM tiles
                # (single-tile-config groups), overlapped at (0,0)/(64,0);
                # merged on DVE. Bias rides in y_lo's group via ones64/bias64.
                y_lo = y_psp.tile([128, D], F32, name="y_lo")
                y_hi = y_psp.tile([128, D], F32, name="y_hi")
                tsl = slice(128 * t, 128 * (t + 1))
                for ki, kc in enumerate(KC_ORDER):
                    cl = ctxall[kc]
                    st = (ki == 0)
                    for fs, fe in ((0, 512), (512, D)):
                        nc.tensor.matmul(
                            y_lo[:, fs:fe], cl[0:64, tsl],
                            wp_sb[kc][0:64, fs:fe],
                            start=st, stop=False, tile_position=(0, 0))
                        nc.tensor.matmul(
                            y_hi[:, fs:fe], cl[64:128, tsl],
                            wp_sb[kc][64:128, fs:fe],
                            start=st, stop=(ki == KC - 1),
                            tile_position=(64, 0))
                for fs, fe in ((0, 512), (512, D)):
                    nc.tensor.matmul(
                        y_lo[:, fs:fe], ones64[:], bias64[:, fs:fe],
                        start=False, stop=True, tile_position=(0, 0))
                y_sb = y_sbp.tile([128, D], F32, name="y_sb")
                nc.vector.tensor_copy(y_sb[:], y_lo[:])
                nc.vector.tensor_add(y_sb[:], y_hi[:], y_sb[:])
                nc.sync.dma_start(
                    out_y[QT * bb + 128 * t:QT * bb + 128 * (t + 1), :],
                    y_sb[:])

    persist.release()
    dram.release()
    ctx_lp.__exit__(None, None, None)


def shard_inputs(x, W_kqv, W_proj, b_proj, cfg: Cfg):
    """Full inputs -> list of 8 per-core input dicts (numpy, host layout)."""
    HD = cfg.HD
    in_maps = []
    x = np.asarray(x, np.float32)
    W_kqv = np.asarray(W_kqv, np.float32)
    wpt = np.ascontiguousarray(
        np.asarray(W_proj, np.float32).T).astype(ml_dtypes.bfloat16)
    bias = np.ascontiguousarray(
        np.asarray(b_proj, np.float32).reshape(1, cfg.D))
    for c in range(cfg.NCORES):
        b = c // 4
        g = c % 4
        hs = [3 * g, 3 * g + 1, 3 * g + 2]
        k = [W_kqv[h][:, 0:HD] for h in hs]
        q = [W_kqv[h][:, HD:2 * HD] for h in hs]
        v = [W_kqv[h][:, 2 * HD:3 * HD] for h in hs]
        wkv = np.concatenate(
            [k[0], k[1], q[0], q[1], k[2], q[2],
             v[0], v[1], v[2]], axis=1).astype(np.float32)
        in_maps.append({
            "xT": np.ascontiguousarray(x[b].T).astype(ml_dtypes.bfloat16),
            "wkv": np.ascontiguousarray(wkv).astype(ml_dtypes.bfloat16),
            "wpt": wpt,
            "bias": bias,
        })
    return in_maps


def assemble_output(outs, cfg: Cfg):
    """Per-core y [2*QT, D] -> full [B, N, D]."""
    y = np.zeros((cfg.B, cfg.N, cfg.D), np.float32)
    for c in range(cfg.NCORES):
        o = outs[c]
        for bb in range(cfg.B):
            y[bb, cfg.QT * c:cfg.QT * (c + 1), :] = (
                o[cfg.QT * bb:cfg.QT * (bb + 1), :])
    return y


_NC_CACHE = {}


def _build_nc(cfg):
    from concourse import bacc

    nc = bacc.Bacc(
        "TRN2", target_bir_lowering=False, debug=False,
        num_devices=cfg.NCORES)
    ins = {
        "xT": nc.dram_tensor("xT", [cfg.D, cfg.N], BF16,
                             kind="ExternalInput").ap(),
        "wkv": nc.dram_tensor("wkv", [cfg.D, 576], BF16,
                              kind="ExternalInput").ap(),
        "wpt": nc.dram_tensor("wpt", [cfg.D, cfg.D], BF16,
                              kind="ExternalInput").ap(),
        "bias": nc.dram_tensor("bias", [1, cfg.D], F32,
                               kind="ExternalInput").ap(),
    }
    out = nc.dram_tensor("y", [2 * cfg.QT, cfg.D], F32,
                         kind="ExternalOutput").ap()
    with tile.TileContext(nc) as tc:
        build(tc, out, ins, cfg)
    nc.compile()
    return nc


def _get_nc(cfg):
    if "nc" not in _NC_CACHE:
        _NC_CACHE["nc"] = _build_nc(cfg)
    return _NC_CACHE["nc"]


def run_sharded(inputs, trace=False):
    import concourse.bass_utils as bass_utils

    cfg = Cfg(N=4096)
    in_maps = shard_inputs(
        inputs["x"], inputs["W_kqv"], inputs["W_proj"], inputs["b_proj"], cfg)
    nc = _get_nc(cfg)
    res = bass_utils.run_bass_kernel_spmd(
        nc, in_maps, core_ids=list(range(cfg.NCORES)), trace=trace)
    outs = [res.results[c]["y"] for c in range(cfg.NCORES)]
    return assemble_output(outs, cfg), res


def kernel(**inputs):
    y, _ = run_sharded(inputs, trace=False)
    return y


# revision 43
# speedup vs baseline: 1.0468x; 1.0296x over previous
"""Distributed causal self-attention kernel for 8 TRN2 NeuronCores (Bass/Tile).

Self-contained: kernel(**inputs) takes the FULL unsharded inputs
(x [2,4096,768], W_kqv [12,768,192], W_proj [768,768], b_proj [768]),
shards them across 8 cores (batch x head-group), runs one SPMD NEFF via
bass_utils.run_bass_kernel_spmd, and reassembles the full [2,4096,768] output.

v2: software-pipelined phases — QKV projection interleaved with A/B-head
attention per token-half so scalar-engine exp overlaps projection matmuls;
V projected directly into [keys, vdim] layout (no PE transposes); PV
stationaries are 128-col (FWL); fast approximate reciprocal; head-C pass
runs under the A/B all-to-alls so only the small head-C collective is
tail-exposed.
"""

import sys

for p in ("/opt/trn_rl_repo", "/root/.axon_site/_ro/trn_rl_repo"):
    if p not in sys.path:
        sys.path.insert(0, p)


import ml_dtypes
import numpy as np

import concourse.bass as bass
import concourse.mybir as mybir
import concourse.tile as tile

F32 = mybir.dt.float32
F32R = mybir.dt.float32r
BF16 = mybir.dt.bfloat16
EXPF = mybir.ActivationFunctionType.Exp


class Cfg:
    def __init__(self, N=4096, D=768, H=12, B=2, NCORES=8):
        self.N, self.D, self.H, self.B, self.NCORES = N, D, H, B, NCORES
        self.HD = D // H          # 64
        self.HPC = H // (NCORES // B)   # heads per core = 3
        self.KB = 128             # k block
        self.NKB = N // self.KB   # k blocks (32)
        self.QT = N // NCORES     # q tile == per-rank token chunk (512)
        self.R = self.QT // self.KB  # diag masks per q tile (4)
        self.KC = D // 128        # contraction chunks (6)
        self.VW = 65              # v block cols: 64 v dims + ones col
        self.VS = 3 * self.VW     # v_sb stride per key block (195)
        assert self.HD == 64 and self.HPC == 3 and self.QT % self.KB == 0


def r32(ap):
    return ap.bitcast(F32R) if ap.dtype == F32 else ap


def build(tc: tile.TileContext, out_y: bass.AP, ins: dict, cfg: Cfg):
    nc = tc.nc
    ctx_lp = nc.allow_low_precision(reason="fp32r matmul pipeline")
    ctx_lp.__enter__()
    N, D, QT, KB, R, KC, NKB = cfg.N, cfg.D, cfg.QT, cfg.KB, cfg.R, cfg.KC, cfg.NKB
    HD, VW, VS = cfg.HD, cfg.VW, cfg.VS
    scale = 1.0 / np.sqrt(HD)
    xT, wkv, wpt, bias = ins["xT"], ins["wkv"], ins["wpt"], ins["bias"]

    persist = tc.alloc_tile_pool(name="persist", bufs=1)
    const_p = persist

    # ---- weights + x. Few, large DMAs: the sync engine issues descriptors
    # serially (~0.6us each), so batching directly shortens startup.
    wkv_sb = []   # [kA|kB | qA|qB | kC|qC | vA|vB|vC] per contraction chunk
    for kc in range(KC):
        w = const_p.tile([128, 576], BF16, name=f"wkv{kc}")
        nc.sync.dma_start(w[:], wkv[128 * kc:128 * (kc + 1), :])
        wkv_sb.append(w)
    xt_sb = []
    for kc in range(KC):
        x = const_p.tile([128, N], BF16, name=f"xt{kc}")
        nc.sync.dma_start(x[:, 0:1024], xT[128 * kc:128 * (kc + 1), 0:1024])
        xt_sb.append(x)
    bias_sb = const_p.tile([1, D], F32R)
    nc.sync.dma_start(bias_sb[:], bias[:].bitcast(F32R))
    # rest of x + the phase-D weights in the background
    for half in range(1, 4):
        for kc in range(KC):
            nc.sync.dma_start(
                xt_sb[kc][:, 1024 * half:1024 * (half + 1)],
                xT[128 * kc:128 * (kc + 1), 1024 * half:1024 * (half + 1)])
    wp_sb = []
    for kc in range(KC):
        w = const_p.tile([128, D], BF16, name=f"wp{kc}")
        nc.sync.dma_start(w[:], wpt[128 * kc:128 * (kc + 1), :])
        wp_sb.append(w)

    # diag masks: mask_d[p, c] = 1.0 if c >= KB*d + p else 0
    masks = []
    for d in range(R):
        mk = const_p.tile([128, QT], BF16, name=f"mask{d}")
        nc.vector.memset(mk[:], 1.0)
        nc.gpsimd.affine_select(
            out=mk[:], in_=mk[:], compare_op=mybir.AluOpType.is_ge,
            fill=0.0, base=-KB * d, pattern=[[1, QT]], channel_multiplier=-1)
        masks.append(mk)

    ones128 = const_p.tile([1, 128], F32R)
    onestage = const_p.tile([1, 128], F32)
    nc.vector.memset(onestage[:], 1.0)
    nc.vector.tensor_copy(ones128[:], onestage[:])
    # 64-partition ones / zero-padded bias pair: lets the phase-D bias add run
    # as a (64,128)-tile matmul matching the lo-half group's tile config
    ones64 = const_p.tile([64, 128], BF16)
    nc.vector.memset(ones64[:], 1.0)
    ones65_f = const_p.tile([65, 64], F32)
    nc.vector.memset(ones65_f[:], 1.0)
    ones65 = const_p.tile([65, 64], F32R)
    nc.vector.tensor_copy(ones65[:], ones65_f[:])
    bias64 = const_p.tile([64, D], BF16)
    nc.vector.memset(bias64[:], 0.0)
    nc.vector.tensor_copy(bias64[0:1, :], bias_sb[:].bitcast(F32))

    # v in [keys, vdim] layout: per key block j cols VS*j + VW*hi + [0:64]
    # are head hi's v, col VS*j + VW*hi + 64 is the ones column. Tail pad so
    # 128-col stationary reads (FWL) stay in bounds. memset 1.0 fills the
    # ones columns; v copies overwrite the data columns.
    v_sb = const_p.tile([128, VS * NKB + 128], BF16)
    # quarter memsets so early v copies only wait on their own region
    VQ = (VS * NKB + 128) // 4
    for q in range(4):
        hi_col = VS * NKB + 128 if q == 3 else VQ * (q + 1)
        nc.vector.memset(v_sb[:, VQ * q:hi_col], 1.0)

    # persistent kq activations
    kq_ab = const_p.tile([128, 2 * N], BF16)   # p0:64 kA|qA, p64:128 kB|qB
    kq_ck = const_p.tile([128, N], BF16)       # kC duplicated in both halves
    kq_cq = const_p.tile([128, N], BF16)       # qC duplicated in both halves

    dram = tc.alloc_tile_pool(name="dram", bufs=1, space="DRAM")
    a2a_in = []
    a2a_out = []
    for hi in range(3):
        ai = dram.tile([cfg.NCORES, 64, QT], BF16, name=f"a2ain{hi}")
        ao = dram.tile([cfg.NCORES, 64, QT], BF16, name=f"a2aout{hi}")
        a2a_in.append(ai)
        a2a_out.append(ao)

    def k_slice(hi, j):
        if hi == 0:
            return kq_ab[0:64, KB * j:KB * (j + 1)]
        if hi == 1:
            return kq_ab[64:128, KB * j:KB * (j + 1)]
        return kq_ck[0:64, KB * j:KB * (j + 1)]

    def q_slice(hi, qt):
        if hi == 0:
            return kq_ab[0:64, N + QT * qt:N + QT * (qt + 1)]
        if hi == 1:
            return kq_ab[64:128, N + QT * qt:N + QT * (qt + 1)]
        return kq_cq[0:64, QT * qt:QT * (qt + 1)]

    def v_stat(hi, j):
        # 128-col stationary: cols 0:64 = v, col 64 = ones, rest junk
        return v_sb[:, VS * j + VW * hi:VS * j + VW * hi + 128]

    with (
        tc.tile_pool(name="scr_ps", bufs=2, space="PSUM") as scr_psp,
        tc.tile_pool(name="s_ps", bufs=2, space="PSUM") as s_psp,
        tc.tile_pool(name="ctx_ps", bufs=1, space="PSUM") as ctx_psp,
        tc.tile_pool(name="exp_sb", bufs=4) as exp_sbp,
        tc.tile_pool(name="small_sb", bufs=3) as small_p,
        tc.tile_pool(name="cn_sb", bufs=3) as cn_p,
    ):
        def kq_tile(mt, gch):
            fr = slice(512 * gch, 512 * (gch + 1))
            ps = scr_psp.tile([128, 512], F32, name="scr")
            for kc in range(KC):
                nc.tensor.matmul(
                    ps[:], wkv_sb[kc][:, 128 * mt:128 * (mt + 1)],
                    xt_sb[kc][:, fr],
                    start=(kc == 0), stop=(kc == KC - 1))
            if mt == 0:
                nc.vector.tensor_copy(kq_ab[:, fr], ps[:])
            elif mt == 1:
                nc.vector.tensor_copy(
                    kq_ab[:, N + 512 * gch:N + 512 * (gch + 1)], ps[:])
            else:
                # mt 2 = [kC | qC] computed once; duplicate into both
                # partition halves (32-partition copies route freely
                # across quadrants)
                nc.vector.tensor_copy(kq_ck[0:64, fr], ps[0:64, :])
                nc.vector.tensor_copy(kq_ck[64:96, fr], ps[0:32, :])
                nc.vector.tensor_copy(kq_ck[96:128, fr], ps[32:64, :])
                nc.vector.tensor_copy(kq_cq[64:128, fr], ps[64:128, :])
                nc.vector.tensor_copy(kq_cq[0:32, fr], ps[64:96, :])
                nc.vector.tensor_copy(kq_cq[32:64, fr], ps[96:128, :])

        def v_tile(j):
            tfr = slice(128 * j, 128 * (j + 1))
            ps = scr_psp.tile([128, 512], F32, name="scr")
            for kc in range(KC):
                nc.tensor.matmul(
                    ps[:, 0:192], xt_sb[kc][:, tfr], wkv_sb[kc][:, 384:576],
                    start=(kc == 0), stop=(kc == KC - 1))
            for hi in range(3):
                nc.vector.tensor_copy(
                    v_sb[:, VS * j + VW * hi:VS * j + VW * hi + 64],
                    ps[:, 64 * hi:64 * (hi + 1)])

        def a_chunk_tiles(gch):
            """Projection work for one 512-token chunk as 8 filler closures."""
            fillers = [
                (lambda mt=mt: kq_tile(mt, gch)) for mt in range(3)]
            fillers += [
                (lambda j=j: v_tile(j)) for j in range(4 * gch, 4 * gch + 4)]
            return fillers

        def recip_row(ctx_ps, dst):
            # custom DVE ops misread partition-offset inputs: stage the sums
            # row at partition 0 before the approx reciprocal
            s_f32 = small_p.tile([1, QT], F32, name="s_f32")
            nc.vector.tensor_copy(s_f32[:], ctx_ps[64:65, :])
            r_f32 = small_p.tile([1, QT], F32, name="r_f32")
            nc.vector.reciprocal_approx_fast(out=r_f32[:], in_=s_f32[:])
            nc.vector.tensor_copy(dst, r_f32[:])

        def bc_and_ship(hi, qt, ctx_ps, bc_ps):
            bc_sb = small_p.tile([64, QT], F32, name="bc_sb")
            nc.vector.tensor_copy(bc_sb[:], bc_ps[0:64, :])
            cn = cn_p.tile([64, QT], BF16, name="cn")
            nc.vector.tensor_mul(cn[:], ctx_ps[0:64, :], bc_sb[:])
            nc.sync.dma_start(a2a_in[hi][qt], cn[:])

        def norm_and_ship_AB(qt, ctxA, ctxB):
            # reciprocals staged at partitions 0 (head A) and 64 (head B) so
            # the two broadcast matmuls pair across PE row-tiles
            r_pair = small_p.tile([65, QT], F32R, name="r_pair")
            recip_row(ctxA, r_pair[0:1, :])
            recip_row(ctxB, r_pair[64:65, :])
            bcA = scr_psp.tile([128, 512], F32, name="scr")
            bcB = scr_psp.tile([128, 512], F32, name="scr")
            nc.tensor.matmul(
                bcA[0:64, :], ones65[0:1, :], r_pair[0:1, :],
                start=True, stop=True, tile_position=(0, 0))
            nc.tensor.matmul(
                bcB[0:64, :], ones65[64:65, :], r_pair[64:65, :],
                start=True, stop=True, tile_position=(64, 0))
            bc_and_ship(0, qt, ctxA, bcA)
            bc_and_ship(1, qt, ctxB, bcB)

        def norm_and_ship(hi, qt, ctx_ps):
            r_sb = small_p.tile([1, QT], F32R, name="r_sb")
            recip_row(ctx_ps, r_sb[:])
            bc_ps = scr_psp.tile([128, 512], F32, name="scr")
            nc.tensor.matmul(
                bc_ps[0:64, :], ones128[:, 0:64], r_sb[:],
                start=True, stop=True)
            bc_and_ship(hi, qt, ctx_ps, bc_ps)

        def a2a(hi):
            nc.gpsimd.collective_compute(
                "AllToAll", mybir.AluOpType.bypass,
                replica_groups=[list(range(cfg.NCORES))],
                ins=[a2a_in[hi].opt()], outs=[a2a_out[hi].opt()])

        def issue_AB(qt, fillers=()):
            """A/B-head attention q-tile; `fillers` are projection-tile
            closures sprinkled between j blocks to keep scalar fed.

            Diagonal blocks (d >= 1) trim S/exp/PV to the valid query range
            [128d, QT); the last block runs PV full-width (mask zeroes the
            stale region) so every PSUM column's accumulation group gets its
            stop flag."""
            njb = (qt + 1) * R
            ctxA = ctx_psp.tile([128, QT], F32, name="ctxA", tag="ctxA", bufs=1)
            ctxB = ctx_psp.tile([128, QT], F32, name="ctxB", tag="ctxB", bufs=1)
            nf = len(fillers)
            k = 0
            for j in range(njb):
                while k < nf and k * njb <= j * nf:
                    fillers[k]()
                    k += 1
                d = j - R * qt
                trim = 128 * d if d >= 1 else 0
                last = (j == njb - 1)
                s_ps = s_psp.tile([128, 2 * QT], F32, name="s_ps")
                for hh, (hi, po) in enumerate(((0, 0), (1, 64))):
                    nc.tensor.matmul(
                        s_ps[:, QT * hh + trim:QT * (hh + 1)],
                        k_slice(hi, j),
                        q_slice(hi, qt)[:, trim:QT],
                        start=True, stop=True, tile_position=(po, 0))
                ex = exp_sbp.tile([128, 2 * QT], BF16, name="ex")
                if trim:
                    for hh in range(2):
                        nc.scalar.activation(
                            ex[:, QT * hh + trim:QT * (hh + 1)],
                            s_ps[:, QT * hh + trim:QT * (hh + 1)],
                            EXPF, scale=scale)
                else:
                    nc.scalar.activation(ex[:], s_ps[:], EXPF, scale=scale)
                if d >= 0:
                    moff = 0 if last else trim
                    for hh in range(2):
                        nc.vector.tensor_mul(
                            ex[:, QT * hh + moff:QT * (hh + 1)],
                            ex[:, QT * hh + moff:QT * (hh + 1)],
                            masks[d][:, moff:QT])
                pvoff = 0 if last else trim
                nc.tensor.matmul(
                    ctxA[:, pvoff:QT], v_stat(0, j), ex[:, pvoff:QT],
                    start=(j == 0), stop=last)
                nc.tensor.matmul(
                    ctxB[:, pvoff:QT], v_stat(1, j),
                    ex[:, QT + pvoff:2 * QT],
                    start=(j == 0), stop=last)
            while k < nf:
                fillers[k]()
                k += 1
            norm_and_ship_AB(qt, ctxA, ctxB)

        def issue_C(qt):
            # njb is always even: even j blocks accumulate into ctxCe, odd
            # into ctxCo (cross-paired over both array halves); merged at ship
            njb = (qt + 1) * R
            ctxCe = ctx_psp.tile([128, QT], F32, name="ctxCe", tag="ctxA",
                                 bufs=1)
            ctxCo = ctx_psp.tile([128, QT], F32, name="ctxCo", tag="ctxB",
                                 bufs=1)
            for jg in range(0, njb, 2):
                je, jo = jg, jg + 1
                regions = []
                for i, (j, acc) in enumerate(((je, ctxCe), (jo, ctxCo))):
                    d = j - R * qt
                    st = (j == i)                    # accumulator start block
                    sp = (j == njb - 2 + i)          # accumulator stop block
                    trim = 128 * d if d >= 1 else 0
                    pvoff = 0 if (st or sp or d < 1) else trim
                    regions.append((i, j, acc, d, st, sp, trim, pvoff))
                s_ps = s_psp.tile([128, 2 * QT], F32, name="s_ps")
                for i, j, acc, d, st, sp, trim, pvoff in regions:
                    po = 64 * i
                    nc.tensor.matmul(
                        s_ps[:, QT * i + trim:QT * (i + 1)],
                        kq_ck[po:po + 64, KB * j:KB * (j + 1)],
                        kq_cq[po:po + 64,
                              QT * qt + trim:QT * (qt + 1)],
                        start=True, stop=True, tile_position=(64 * i, 0))
                ex = exp_sbp.tile([128, 2 * QT], BF16, name="ex")
                if any(r[6] for r in regions):
                    for i, j, acc, d, st, sp, trim, pvoff in regions:
                        nc.scalar.activation(
                            ex[:, QT * i + trim:QT * (i + 1)],
                            s_ps[:, QT * i + trim:QT * (i + 1)],
                            EXPF, scale=scale)
                else:
                    nc.scalar.activation(ex[:], s_ps[:], EXPF, scale=scale)
                for i, j, acc, d, st, sp, trim, pvoff in regions:
                    if d >= 0:
                        nc.vector.tensor_mul(
                            ex[:, QT * i + pvoff:QT * (i + 1)],
                            ex[:, QT * i + pvoff:QT * (i + 1)],
                            masks[d][:, pvoff:QT])
                for i, j, acc, d, st, sp, trim, pvoff in regions:
                    nc.tensor.matmul(
                        acc[:, pvoff:QT], v_stat(2, j),
                        ex[:, QT * i + pvoff:QT * (i + 1)],
                        start=st, stop=sp)
            # merge even/odd accumulators and ship
            sE = small_p.tile([1, QT], F32, name="sE")
            nc.vector.tensor_copy(sE[:], ctxCe[64:65, :])
            s_f32 = small_p.tile([1, QT], F32, name="s_f32")
            nc.vector.tensor_add(s_f32[:], ctxCo[64:65, :], sE[:])
            r_f32 = small_p.tile([1, QT], F32, name="r_f32")
            nc.vector.reciprocal_approx_fast(out=r_f32[:], in_=s_f32[:])
            r_sb = small_p.tile([1, QT], F32R, name="r_sb")
            nc.vector.tensor_copy(r_sb[:], r_f32[:])
            bc_ps = scr_psp.tile([128, 512], F32, name="scr")
            nc.tensor.matmul(
                bc_ps[0:64, :], ones128[:, 0:64], r_sb[:],
                start=True, stop=True)
            bc_sb = small_p.tile([64, QT], F32, name="bc_sb")
            nc.vector.tensor_copy(bc_sb[:], bc_ps[0:64, :])
            cE = cn_p.tile([64, QT], F32, name="cE")
            nc.vector.tensor_copy(cE[:], ctxCe[0:64, :])
            cS = cn_p.tile([64, QT], F32, name="cS")
            nc.vector.tensor_add(cS[:], ctxCo[0:64, :], cE[:])
            cn = cn_p.tile([64, QT], BF16, name="cn")
            nc.vector.tensor_mul(cn[:], cS[:], bc_sb[:])
            nc.sync.dma_start(a2a_in[2][qt], cn[:])

        # -------- pipelined issue --------
        # chunk 0 projected up front; chunk qt+1's projection tiles fill the
        # gaps inside attention q-tile qt (which only needs chunks <= qt)
        for f in a_chunk_tiles(0):
            f()
        for qt in range(N // QT):
            fillers = a_chunk_tiles(qt + 1) if qt + 1 < N // QT else ()
            issue_AB(qt, fillers)
        a2a(0)
        a2a(1)
        for qt in range(N // QT):
            issue_C(qt)
        a2a(2)

    # ---------------- Phase D: output projection ----------------
    KC_ORDER = [0, 3, 1, 2, 4, 5]   # kc needing only heads A/B first
    with (
        tc.tile_pool(name="ctxall", bufs=1) as ctxall_p,
        tc.tile_pool(name="y_ps", bufs=2, space="PSUM") as y_psp,
        tc.tile_pool(name="y_sb", bufs=3) as y_sbp,
    ):
        for bb in range(cfg.B):
            ctxall = [None] * KC
            for kc in KC_ORDER:
                ct = ctxall_p.tile([128, QT], BF16, name=f"ctxall{bb}_{kc}")
                for sub in range(2):
                    h = 2 * kc + sub
                    g, hi = h // 3, h % 3
                    nc.sync.dma_start(
                        ct[64 * sub:64 * (sub + 1), :],
                        a2a_out[hi][4 * bb + g])
                ctxall[kc] = ct
            for t in range(QT // 128):
                # lo/hi contraction halves accumulate in separate PSUM tiles
                # (single-tile-config groups), overlapped at (0,0)/(64,0);
                # merged on DVE. Bias rides in y_lo's group via ones64/bias64.
                y_lo = y_psp.tile([128, D], F32, name="y_lo")
                y_hi = y_psp.tile([128, D], F32, name="y_hi")
                tsl = slice(128 * t, 128 * (t + 1))
                for ki, kc in enumerate(KC_ORDER):
                    cl = ctxall[kc]
                    st = (ki == 0)
                    for fs, fe in ((0, 512), (512, D)):
                        nc.tensor.matmul(
                            y_lo[:, fs:fe], cl[0:64, tsl],
                            wp_sb[kc][0:64, fs:fe],
                            start=st, stop=False, tile_position=(0, 0))
                        nc.tensor.matmul(
                            y_hi[:, fs:fe], cl[64:128, tsl],
                            wp_sb[kc][64:128, fs:fe],
                            start=st, stop=(ki == KC - 1),
                            tile_position=(64, 0))
                for fs, fe in ((0, 512), (512, D)):
                    nc.tensor.matmul(
                        y_lo[:, fs:fe], ones64[:], bias64[:, fs:fe],
                        start=False, stop=True, tile_position=(0, 0))
                y_sb = y_sbp.tile([128, D], F32, name="y_sb")
                nc.vector.tensor_copy(y_sb[:], y_lo[:])
                nc.vector.tensor_add(y_sb[:], y_hi[:], y_sb[:])
                nc.sync.dma_start(
                    out_y[QT * bb + 128 * t:QT * bb + 128 * (t + 1), :],
                    y_sb[:])

    persist.release()
    dram.release()
    ctx_lp.__exit__(None, None, None)


def shard_inputs(x, W_kqv, W_proj, b_proj, cfg: Cfg):
    """Full inputs -> list of 8 per-core input dicts (numpy, host layout)."""
    HD = cfg.HD
    in_maps = []
    x = np.asarray(x, np.float32)
    W_kqv = np.asarray(W_kqv, np.float32)
    wpt = np.ascontiguousarray(
        np.asarray(W_proj, np.float32).T).astype(ml_dtypes.bfloat16)
    bias = np.ascontiguousarray(
        np.asarray(b_proj, np.float32).reshape(1, cfg.D))
    for c in range(cfg.NCORES):
        b = c // 4
        g = c % 4
        hs = [3 * g, 3 * g + 1, 3 * g + 2]
        k = [W_kqv[h][:, 0:HD] for h in hs]
        q = [W_kqv[h][:, HD:2 * HD] for h in hs]
        v = [W_kqv[h][:, 2 * HD:3 * HD] for h in hs]
        wkv = np.concatenate(
            [k[0], k[1], q[0], q[1], k[2], q[2],
             v[0], v[1], v[2]], axis=1).astype(np.float32)
        in_maps.append({
            "xT": np.ascontiguousarray(x[b].T).astype(ml_dtypes.bfloat16),
            "wkv": np.ascontiguousarray(wkv).astype(ml_dtypes.bfloat16),
            "wpt": wpt,
            "bias": bias,
        })
    return in_maps


def assemble_output(outs, cfg: Cfg):
    """Per-core y [2*QT, D] -> full [B, N, D]."""
    y = np.zeros((cfg.B, cfg.N, cfg.D), np.float32)
    for c in range(cfg.NCORES):
        o = outs[c]
        for bb in range(cfg.B):
            y[bb, cfg.QT * c:cfg.QT * (c + 1), :] = (
                o[cfg.QT * bb:cfg.QT * (bb + 1), :])
    return y


_NC_CACHE = {}


def _build_nc(cfg):
    from concourse import bacc

    nc = bacc.Bacc(
        "TRN2", target_bir_lowering=False, debug=False,
        num_devices=cfg.NCORES)
    ins = {
        "xT": nc.dram_tensor("xT", [cfg.D, cfg.N], BF16,
                             kind="ExternalInput").ap(),
        "wkv": nc.dram_tensor("wkv", [cfg.D, 576], BF16,
                              kind="ExternalInput").ap(),
        "wpt": nc.dram_tensor("wpt", [cfg.D, cfg.D], BF16,
                              kind="ExternalInput").ap(),
        "bias": nc.dram_tensor("bias", [1, cfg.D], F32,
                               kind="ExternalInput").ap(),
    }
    out = nc.dram_tensor("y", [2 * cfg.QT, cfg.D], F32,
                         kind="ExternalOutput").ap()
    with tile.TileContext(nc) as tc:
        build(tc, out, ins, cfg)
    nc.compile()
    return nc


def _get_nc(cfg):
    if "nc" not in _NC_CACHE:
        _NC_CACHE["nc"] = _build_nc(cfg)
    return _NC_CACHE["nc"]


def run_sharded(inputs, trace=False):
    import concourse.bass_utils as bass_utils

    cfg = Cfg(N=4096)
    in_maps = shard_inputs(
        inputs["x"], inputs["W_kqv"], inputs["W_proj"], inputs["b_proj"], cfg)
    nc = _get_nc(cfg)
    res = bass_utils.run_bass_kernel_spmd(
        nc, in_maps, core_ids=list(range(cfg.NCORES)), trace=trace)
    outs = [res.results[c]["y"] for c in range(cfg.NCORES)]
    return assemble_output(outs, cfg), res


def kernel(**inputs):
    y, _ = run_sharded(inputs, trace=False)
    return y
